# revision 17
# baseline (speedup 1.0000x reference)
"""CenterLoss Trainium2 kernel (label-sorted data-parallel over 8 NeuronCores).

loss = sum(clip(distmat * onehot(labels), 1e-12, 1e12)) / B,
distmat[i,c] = ||x_i - centers_c||^2. Only the (i, labels_i) entries survive
the mask; the B*(C-1) masked entries contribute exactly 1e-12 each (added
analytically on host). d_i ~ 4096 so the clip never binds and

  sum_i d_i = sum_i ||x_i||^2 + sum_c n_c ||c_c||^2 - 2 sum_c <s_c, c_c>

with s = onehot(labels)^T @ x and n = histogram(labels).

Sharding: samples are SORTED BY LABEL on host (a pure permutation — sum_i is
permutation-invariant) and split into 8 equal 1024-sample shards. Each
shard's labels then span < 112 consecutive classes (seed-0 spans are 90-99),
so each core loads only a 112-row window of centers (0.9 MB) instead of all
751 rows (6.15 MB). Per-core HBM traffic is 9.3 MB, near the 8.4 MB
cost-model floor for streaming the fp32 x shard at the simulator's
serialized 360 B/ns DMA bandwidth.

The class window collapses the one-hot matmul to a single class tile: s is
[112, 2048], computed on the PE in fp32r (TF32-style; 1 cycle/row at
>=256-wide outputs, ~1.4e-4 rel vs fp32 on HW) directly from the DMA-loaded
x tiles bitcast to fp32r — no fp8/bf16 conversion pass. ||x||^2 and ||c||^2
are exact f32 (scalar-engine Square with accum, with some late pieces as
x*x scalar_tensor_tensor on DVE/GPSIMD); -2<s,c> is fused into the PSUM
drain via DVE scalar_tensor_tensor (GPSIMD cannot access PSUM).

Scheduling: the DMA stream (the bottleneck: all transfers serialize at
360 B/ns) carries x tiles 0-4 whole, tiles 5-6 as halves, and tile 7 as
3 quarters + 2 eighths, so the per-piece squares / closing matmuls / PSUM
drains chase the stream tail at fine granularity instead of serializing
after it. PSUM group A = tiles 0-3 (drained mid-flight), group B = tiles
4-7 (per-bank drains fire as each bank's closing piece lands).

Per-core output is a [128, 28] block of raw partial columns (sum-of-x^2
pieces, drain pieces, and cn2 = ||c_c||^2 in col 27); host combine (f64)
sums everything, adds <histogram, cn2> with each class read from the one
core whose window owns it, adds B*(C-1)*1e-12, and divides by B.
"""

from contextlib import ExitStack

import numpy as np

import concourse.bacc as bacc
import concourse.tile as tile
from concourse import mybir
from concourse.bass_utils import run_bass_kernel_spmd

N_CORES = 8
B = 8192
D = 2048
C = 751
BS = B // N_CORES  # samples per core
P = 128
NT = BS // P       # sample tiles per core
CW = 112           # centers window rows per core (max label span per shard)
NCH = D // 512     # feature chunks (one PSUM bank each)
OUTW = 28
CN2COL = 27
F32 = mybir.dt.float32
F32R = mybir.dt.float32r
CLIP_LO = 1e-12

_NC = None


def build_nc():
    nc = bacc.Bacc("TRN2", target_bir_lowering=False)
    x = nc.dram_tensor("x", [BS, D], F32, kind="ExternalInput")
    labels = nc.dram_tensor("labels", [P, NT], mybir.dt.int32, kind="ExternalInput")
    cwin = nc.dram_tensor("cwin", [CW, D], F32, kind="ExternalInput")
    out = nc.dram_tensor("partial", [P, OUTW], F32, kind="ExternalOutput")

    # x_r[p, t, :] = x[t*128 + p, :]
    x_r = x[:].rearrange("(t p) d -> p t d", p=P)

    with tile.TileContext(nc) as tc, ExitStack() as ctx:
        xp = ctx.enter_context(tc.tile_pool(name="xp", bufs=5))
        sqp = ctx.enter_context(tc.tile_pool(name="sqp", bufs=2))
        perm = ctx.enter_context(tc.tile_pool(name="perm", bufs=1))
        psp = ctx.enter_context(tc.tile_pool(name="psp", bufs=1, space="PSUM"))

        # labels/centers ride the ACT HWDGE ring; x keeps the SP ring
        lab = perm.tile([P, NT], mybir.dt.int32)
        nc.scalar.dma_start(out=lab[:], in_=labels[:])
        lab_f = perm.tile([P, NT], F32)
        nc.vector.tensor_copy(out=lab_f[:], in_=lab[:])

        iota_i = perm.tile([P, CW], mybir.dt.int32)
        nc.gpsimd.iota(iota_i[:], pattern=[[1, CW]], base=0, channel_multiplier=0)
        iota_f = perm.tile([P, CW], F32)
        nc.vector.tensor_copy(out=iota_f[:], in_=iota_i[:])

        out_sb = perm.tile([P, OUTW], F32)
        nc.vector.memset(out_sb[:], 0.0)

        # one-hot lhsT tiles, produced directly as fp32r (0.0/1.0 are exact)
        oh = perm.tile([P, NT, CW], F32R)
        for t in range(NT):
            nc.vector.tensor_scalar(
                out=oh[:, t, :], in0=iota_f[:], scalar1=lab_f[:, t : t + 1],
                scalar2=None, op0=mybir.AluOpType.is_equal,
            )

        ct = perm.tile([CW, D], F32)
        nc.scalar.dma_start(out=ct[:], in_=cwin[:])

        ps = []
        for g in range(2):
            row = []
            for n in range(NCH):
                ps_gn = psp.tile([CW, 512], F32, tag=f"ps{g}_{n}")
                row.append(ps_gn)
            ps.append(row)

        def mm(g, t, lo, hi, stop):
            for n in range(lo, hi):
                nc.tensor.matmul(
                    out=ps[g][n][:], lhsT=oh[:, t, :],
                    rhs=xt[:, n * 512 : (n + 1) * 512],
                    start=(t == 4 * g), stop=stop,
                )

        def sq_act(ap, col):
            s = sqp.tile([P, 2048], F32, tag="sq")
            nc.scalar.activation(
                out=s[:, : ap.shape[-1]], in_=ap.bitcast(F32),
                func=mybir.ActivationFunctionType.Square,
                accum_out=out_sb[:, col : col + 1],
            )

        def sq_stt(eng, ap, col):
            # sum(x*x) on DVE or GPSIMD via (1.0*x)*x with accumulate
            s = sqp.tile([P, 512], F32, tag="stt_o")
            f = ap.bitcast(F32)
            eng.scalar_tensor_tensor(
                out=s[:, : ap.shape[-1]], in0=f, scalar=1.0, in1=f,
                op0=mybir.AluOpType.mult, op1=mybir.AluOpType.mult,
                accum_out=out_sb[:, col : col + 1],
            )

        def drain(g, n, col, lo=0, hi=512):
            s = sqp.tile([CW, 512], F32, tag="stt_o")
            nc.vector.scalar_tensor_tensor(
                out=s[:, : hi - lo], in0=ps[g][n][:, lo:hi], scalar=-2.0,
                in1=ct[:, n * 512 + lo : n * 512 + hi],
                op0=mybir.AluOpType.mult, op1=mybir.AluOpType.mult,
                accum_out=out_sb[:CW, col : col + 1],
            )

        # tiles 0-4: whole loads, whole ACT squares, group A (0-3) / B (4)
        for t in range(5):
            g = t // 4
            xt = xp.tile([P, D], F32R, tag="xt")
            nc.sync.dma_start(out=xt[:], in_=x_r[:, t, :].bitcast(F32R))
            sq_act(xt[:], t)
            mm(g, t, 0, NCH, stop=False)
            if t == 2:
                # cn2 here: ct has landed by now, and the ACT queue is past
                # the early x squares (no head-of-line blocking)
                sqc = sqp.tile([CW, D], F32, tag="sq")
                nc.scalar.activation(
                    out=sqc[:], in_=ct[:],
                    func=mybir.ActivationFunctionType.Square,
                    accum_out=out_sb[:CW, CN2COL : CN2COL + 1],
                )
            if t == 3:
                for n in range(NCH):
                    drain(0, n, 14 + n)

        # tiles 5,6: half loads + half squares (keeps ACT fed at the tail).
        # Each half gets its OWN tile: the Tile framework tracks readers
        # against ALL writers of a tile, so slicing one tile would make every
        # consumer wait for the last half to land.
        for t in (5, 6):
            for h in range(2):
                sl = slice(h * 1024, (h + 1) * 1024)
                xt = xp.tile([P, 1024], F32R, tag="xh")
                # alternate HWDGE rings so consecutive PE data-waits reference
                # different semaphores (same-sem waits get merged to the
                # later value, serializing the tail)
                ring = nc.sync if h == 0 else nc.scalar
                ring.dma_start(out=xt[:], in_=x_r[:, t, sl].bitcast(F32R))
                sq_act(xt[:], 5 + 2 * (t - 5) + h)
                for n in (2 * h, 2 * h + 1):
                    nc.tensor.matmul(
                        out=ps[1][n][:], lhsT=oh[:, t, :],
                        rhs=xt[:, n * 512 - h * 1024 : (n + 1) * 512 - h * 1024],
                        start=False, stop=(t == 6),
                    )
                if t == 6:
                    # banks close at tile 6 (tile-7 pieces use fresh banks),
                    # so these drains run under the remaining stream
                    drain(1, 2 * h, 14 + 4 + 2 * h)
                    drain(1, 2 * h + 1, 14 + 4 + 2 * h + 1)

        # tile 7: 3 quarters + 2 eighths, each its own tile and its own
        # single-matmul PSUM accumulation (reusing group A's drained banks):
        # every piece's matmul + drain + square fires the moment it lands.
        t7 = [(0, 512), (512, 1024), (1024, 1536), (1536, 1792), (1792, 2048)]
        sq_eng = [None, None, nc.vector, nc.vector, None]  # None -> ACT
        for i, (lo, hi) in enumerate(t7):
            ring = nc.scalar if i % 2 else nc.sync
            xt = xp.tile([P, hi - lo], F32R, tag="xq")
            ring.dma_start(out=xt[:], in_=x_r[:, 7, lo:hi].bitcast(F32R))
            psq = ps[0][i % NCH]
            nc.tensor.matmul(
                out=psq[:, : hi - lo], lhsT=oh[:, 7, :], rhs=xt[:],
                start=True, stop=True,
            )
            s = sqp.tile([CW, 512], F32, tag="stt_o")
            nc.vector.scalar_tensor_tensor(
                out=s[:, : hi - lo], in0=psq[:, : hi - lo], scalar=-2.0,
                in1=ct[:, lo:hi],
                op0=mybir.AluOpType.mult, op1=mybir.AluOpType.mult,
                accum_out=out_sb[:CW, 22 + i : 23 + i],
            )
            if sq_eng[i] is None:
                sq_act(xt[:], 9 + i)
            else:
                sq_stt(sq_eng[i], xt[:], 9 + i)

        nc.sync.dma_start(out=out[:], in_=out_sb[:])
    nc.compile()
    return nc


def _shard(x, labels, centers):
    """Sort samples by label, split into 8 equal shards, slice the centers
    window each shard's labels fall in. Returns (in_maps, lo_list)."""
    order = np.argsort(labels, kind="stable")
    xs = x[order]
    ls = labels[order]
    in_maps, los = [], []
    for k in range(N_CORES):
        chunk = ls[k * BS : (k + 1) * BS]
        lo = int(chunk[0])
        assert int(chunk[-1]) - lo < CW, (
            f"shard {k} label span {int(chunk[-1]) - lo + 1} exceeds window {CW}"
        )
        rows = min(CW, C - lo)
        cw = np.zeros((CW, D), dtype=np.float32)
        cw[:rows] = centers[lo : lo + rows]
        # lab[p, t] = local label of sample t*P + p, matching the x tile layout
        lab = np.ascontiguousarray((chunk - lo).astype(np.int32).reshape(NT, P).T)
        in_maps.append({
            "x": np.ascontiguousarray(xs[k * BS : (k + 1) * BS]),
            "labels": lab,
            "cwin": cw,
        })
        los.append(lo)
    return in_maps, los


def make_in_maps(x, labels, centers):
    return _shard(x, labels, centers)[0]


def combine_partials(partials, los, labels):
    total = 0.0
    for p in partials:
        total += float(np.sum(p[:, :CN2COL].astype(np.float64)))
    # n_c * ||c_c||^2: host histogram x device cn2, each class read from the
    # one core whose window owns it (largest k with lo_k <= c)
    hist = np.bincount(np.asarray(labels).astype(np.int64), minlength=C)
    los = np.asarray(los)
    for c in np.nonzero(hist)[0]:
        k = int(np.searchsorted(los, c, side="right")) - 1
        i = int(c) - int(los[k])
        assert 0 <= i < CW
        total += float(hist[c]) * float(partials[k][i, CN2COL])
    total += float(B) * float(C - 1) * CLIP_LO
    return np.array(total / B, dtype=np.float32)


def kernel(**inputs) -> np.ndarray:
    global _NC
    x = np.ascontiguousarray(np.asarray(inputs["x"], dtype=np.float32))
    labels = np.asarray(inputs["labels"]).astype(np.int64)
    centers = np.ascontiguousarray(np.asarray(inputs["centers"], dtype=np.float32))
    assert x.shape == (B, D) and labels.shape == (B,) and centers.shape == (C, D)

    if _NC is None:
        _NC = build_nc()
    in_maps, los = _shard(x, labels, centers)
    res = run_bass_kernel_spmd(_NC, in_maps, core_ids=list(range(N_CORES)))
    return combine_partials([r["partial"] for r in res.results], los, labels)


# revision 39
# speedup vs baseline: 1.1554x; 1.1554x over previous
"""CenterLoss Trainium2 kernel (label-sorted data-parallel over 8 NeuronCores).

loss = sum(clip(distmat * onehot(labels), 1e-12, 1e12)) / B,
distmat[i,c] = ||x_i - centers_c||^2. Only the (i, labels_i) entries survive
the mask; the B*(C-1) masked entries contribute exactly 1e-12 each (added
analytically on host). d_i ~ 4096 so the clip never binds and

  sum_i d_i = sum_i ||x_i||^2 + sum_c n_c ||c_c||^2 - 2 sum_c <s_c, c_c>

with s = onehot(labels)^T @ x and n = histogram(labels).

Sharding: samples are SORTED BY LABEL on host (a pure permutation — sum_i is
permutation-invariant) and split into 8 equal 1024-sample shards. Each
shard's labels then span < 112 consecutive classes (seed-0 spans are 90-99),
so each core loads only a 112-row window of centers (0.9 MB) instead of all
751 rows (6.15 MB). Per-core HBM traffic is 9.3 MB, near the 8.4 MB
cost-model floor for streaming the fp32 x shard at the simulator's
serialized 360 B/ns DMA bandwidth.

The class window collapses the one-hot matmul to a single class tile: s is
[112, 2048], computed on the PE in fp32r (TF32-style; 1 cycle/row at
>=256-wide outputs, ~1.4e-4 rel vs fp32 on HW) directly from the DMA-loaded
x tiles bitcast to fp32r — no fp8/bf16 conversion pass. ||x||^2 and ||c||^2
are exact f32 (scalar-engine Square with accum, with some late pieces as
x*x scalar_tensor_tensor on DVE/GPSIMD); -2<s,c> is fused into the PSUM
drain via DVE scalar_tensor_tensor (GPSIMD cannot access PSUM).

Scheduling: the DMA stream (the bottleneck: all transfers serialize at
360 B/ns) carries x tiles 0-4 whole, tiles 5-6 as halves, and tile 7 as
3 quarters + 2 eighths, so the per-piece squares / closing matmuls / PSUM
drains chase the stream tail at fine granularity instead of serializing
after it. PSUM group A = tiles 0-3 (drained mid-flight), group B = tiles
4-7 (per-bank drains fire as each bank's closing piece lands).

Per-core output is a [128, 28] block of raw partial columns (sum-of-x^2
pieces, drain pieces, and cn2 = ||c_c||^2 in col 27); host combine (f64)
sums everything, adds <histogram, cn2> with each class read from the one
core whose window owns it, adds B*(C-1)*1e-12, and divides by B.
"""

from contextlib import ExitStack

import numpy as np

import concourse.bacc as bacc
import concourse.tile as tile
from concourse import mybir
from concourse.bass_utils import run_bass_kernel_spmd

N_CORES = 8
B = 8192
D = 2048
C = 751
BS = B // N_CORES  # samples per core
P = 128
NT = BS // P       # sample tiles per core
CW = 112           # centers window rows per core (max label span per shard)
NCH = D // 512     # feature chunks (one PSUM bank each)
OUTW = 28
CN2COL = 27
F32 = mybir.dt.float32
F32R = mybir.dt.float32r
CLIP_LO = 1e-12

_NC = None


def build_nc():
    nc = bacc.Bacc("TRN2", target_bir_lowering=False)
    x = nc.dram_tensor("x", [BS, D], F32, kind="ExternalInput")
    labels = nc.dram_tensor("labels", [P, NT], mybir.dt.int32, kind="ExternalInput")
    cwin = nc.dram_tensor("cwin", [CW, D], F32, kind="ExternalInput")
    out = nc.dram_tensor("partial", [P, OUTW], F32, kind="ExternalOutput")

    # x_r[p, t, :] = x[t*128 + p, :]
    x_r = x[:].rearrange("(t p) d -> p t d", p=P)

    with tile.TileContext(nc) as tc, ExitStack() as ctx:
        xp = ctx.enter_context(tc.tile_pool(name="xp", bufs=5))
        sqp = ctx.enter_context(tc.tile_pool(name="sqp", bufs=2))
        perm = ctx.enter_context(tc.tile_pool(name="perm", bufs=1))
        psp = ctx.enter_context(tc.tile_pool(name="psp", bufs=1, space="PSUM"))

        # labels/centers ride the ACT HWDGE ring; x keeps the SP ring
        lab = perm.tile([P, NT], mybir.dt.int32)
        nc.scalar.dma_start(out=lab[:], in_=labels[:])
        lab_f = perm.tile([P, NT], F32)
        nc.vector.tensor_copy(out=lab_f[:], in_=lab[:])

        iota_i = perm.tile([P, CW], mybir.dt.int32)
        nc.gpsimd.iota(iota_i[:], pattern=[[1, CW]], base=0, channel_multiplier=0)
        iota_f = perm.tile([P, CW], F32)
        nc.vector.tensor_copy(out=iota_f[:], in_=iota_i[:])

        out_sb = perm.tile([P, 1, OUTW], F32)
        nc.vector.memset(out_sb[:], 0.0)



        # one-hot lhsT tiles, produced directly as fp32r (0.0/1.0 are exact)
        oh = perm.tile([P, NT, CW], F32R)
        for t in range(NT):
            nc.vector.tensor_scalar(
                out=oh[:, t, :], in0=iota_f[:], scalar1=lab_f[:, t : t + 1],
                scalar2=None, op0=mybir.AluOpType.is_equal,
            )

        ct = perm.tile([CW, D], F32)
        nc.scalar.dma_start(out=ct[:], in_=cwin[:])

        # group A: banks for tiles 0-3 (closed at t3, drained mid-flight).
        # group B: tiles 4-7; chunk 3 is split into two independently-tracked
        # 256-wide psum tiles so the e6/e7 tail pieces close and drain
        # without tile-level WAR coupling.
        ps = []
        for g in range(2):
            row = []
            for n in range(NCH):
                ps_gn = psp.tile([CW, 512], F32, tag=f"ps{g}_{n}")
                row.append(ps_gn)
            ps.append(row)

        def mm(xt, xbase, g, t, lo, hi, start):
            for n in range(lo, hi):
                nc.tensor.matmul(
                    out=ps[g][n][:], lhsT=oh[:, t, :],
                    rhs=xt[:, n * 512 - xbase : (n + 1) * 512 - xbase],
                    start=start, stop=False,
                )

        def sq_act(ap, col):
            s = sqp.tile([P, 2048], F32, tag="sq")
            nc.scalar.activation(
                out=s[:, : ap.shape[-1]], in_=ap.bitcast(F32),
                func=mybir.ActivationFunctionType.Square,
                accum_out=out_sb[:, 0, col : col + 1],
            )

        def sq_stt(eng, ap, col):
            # sum(x*x) on DVE or GPSIMD via (1.0*x)*x with accumulate
            s = sqp.tile([P, 512], F32, tag="stt_o")
            f = ap.bitcast(F32)
            eng.scalar_tensor_tensor(
                out=s[:, : ap.shape[-1]], in0=f, scalar=1.0, in1=f,
                op0=mybir.AluOpType.mult, op1=mybir.AluOpType.mult,
                accum_out=out_sb[:, 0, col : col + 1],
            )

        def drain(pap, dlo, dhi, col):
            s = sqp.tile([CW, 512], F32, tag="stt_o")
            nc.vector.scalar_tensor_tensor(
                out=s[:, : dhi - dlo], in0=pap, scalar=-2.0,
                in1=ct[:, dlo:dhi],
                op0=mybir.AluOpType.mult, op1=mybir.AluOpType.mult,
                accum_out=out_sb[:CW, 0, col : col + 1],
            )

        # tiles 0-4: whole loads, whole ACT squares, group A (0-3) / B (4)
        for t in range(5):
            g = t // 4
            xt = xp.tile([P, D], F32R, tag="xt")
            nc.sync.dma_start(out=xt[:], in_=x_r[:, t, :].bitcast(F32R))
            sq_act(xt[:], t)
            mm(xt, 0, g, t, 0, NCH, start=(t % 4 == 0))
            if t == 2:
                # cn2 here: ct has landed by now, and the ACT queue is past
                # the early x squares (no head-of-line blocking)
                sqc = sqp.tile([CW, D], F32, tag="sq")
                nc.scalar.activation(
                    out=sqc[:], in_=ct[:],
                    func=mybir.ActivationFunctionType.Square,
                    accum_out=out_sb[:CW, 0, CN2COL : CN2COL + 1],
                )
            if t == 3:
                for n in range(NCH):
                    drain(ps[0][n][:], n * 512, (n + 1) * 512, 14 + n)

        # tiles 5,6: half loads + half squares (keeps ACT fed at the tail).
        # Each half gets its OWN tile: the Tile framework tracks readers
        # against ALL writers of a tile, so slicing one tile would make every
        # consumer wait for the last half to land.
        for t in (5, 6):
            for h in range(2):
                sl = slice(h * 1024, (h + 1) * 1024)
                xt = xp.tile([P, 1024], F32R, tag="xh")
                nc.sync.dma_start(out=xt[:], in_=x_r[:, t, sl].bitcast(F32R))
                sq_act(xt[:], 5 + 2 * (t - 5) + h)
                mm(xt, h * 1024, 1, t, 2 * h, 2 * h + 2, start=False)

        # tile 7: 3 quarters + 2 eighths, each its own tile; each piece is the
        # closing matmul of its group-B psum region, so its drain (and square)
        # fires the moment it lands — only one drain trails the last transfer.
        t7 = [(0, 512), (512, 1024), (1024, 1536), (1536, 1792), (1792, 2048)]
        sq_eng = [None, None, nc.vector, None, None]  # None -> ACT
        q2sq_ap = None
        for i, (lo, hi) in enumerate(t7):
            xt = xp.tile([P, hi - lo], F32R, tag="xq")
            nc.sync.dma_start(out=xt[:], in_=x_r[:, 7, lo:hi].bitcast(F32R))
            n = min(i, NCH - 1)
            blo = 0 if i < NCH else 256
            nc.tensor.matmul(
                out=ps[1][n][:, blo : blo + hi - lo], lhsT=oh[:, 7, :],
                rhs=xt[:], start=False, stop=True,
            )
            if i < NCH - 1:
                drain(ps[1][n][:], lo, hi, 18 + i)
            elif i == NCH:
                # one drain for bank 3 after both eighth-matmuls closed it
                drain(ps[1][n][:], 1536, 2048, 18 + i)
            if sq_eng[i] is None:
                sq_act(xt[:], 9 + i)
            else:
                q2sq_ap = (xt, 9 + i)  # deferred: after the bank-3 drain
        # q2's square last on the DVE queue, behind the drains the out waits on
        sq_stt(nc.vector, q2sq_ap[0][:], q2sq_ap[1])

        nc.sync.dma_start(out=out[:], in_=out_sb[:, 0, :])
    nc.compile()
    return nc


def _shard(x, labels, centers):
    """Sort samples by label, split into 8 equal shards, slice the centers
    window each shard's labels fall in. Returns (in_maps, lo_list)."""
    order = np.argsort(labels, kind="stable")
    xs = x[order]
    ls = labels[order]
    in_maps, los = [], []
    for k in range(N_CORES):
        chunk = ls[k * BS : (k + 1) * BS]
        lo = int(chunk[0])
        assert int(chunk[-1]) - lo < CW, (
            f"shard {k} label span {int(chunk[-1]) - lo + 1} exceeds window {CW}"
        )
        rows = min(CW, C - lo)
        cw = np.zeros((CW, D), dtype=np.float32)
        cw[:rows] = centers[lo : lo + rows]
        # lab[p, t] = local label of sample t*P + p, matching the x tile layout
        lab = np.ascontiguousarray((chunk - lo).astype(np.int32).reshape(NT, P).T)
        in_maps.append({
            "x": np.ascontiguousarray(xs[k * BS : (k + 1) * BS]),
            "labels": lab,
            "cwin": cw,
        })
        los.append(lo)
    return in_maps, los


def make_in_maps(x, labels, centers):
    return _shard(x, labels, centers)[0]


def combine_partials(partials, los, labels):
    total = 0.0
    for p in partials:
        total += float(np.sum(p[:, :CN2COL].astype(np.float64)))
    # n_c * ||c_c||^2: host histogram x device cn2, each class read from the
    # one core whose window owns it (largest k with lo_k <= c)
    hist = np.bincount(np.asarray(labels).astype(np.int64), minlength=C)
    los = np.asarray(los)
    for c in np.nonzero(hist)[0]:
        k = int(np.searchsorted(los, c, side="right")) - 1
        i = int(c) - int(los[k])
        assert 0 <= i < CW
        total += float(hist[c]) * float(partials[k][i, CN2COL])
    total += float(B) * float(C - 1) * CLIP_LO
    return np.array(total / B, dtype=np.float32)


def kernel(**inputs) -> np.ndarray:
    global _NC
    x = np.ascontiguousarray(np.asarray(inputs["x"], dtype=np.float32))
    labels = np.asarray(inputs["labels"]).astype(np.int64)
    centers = np.ascontiguousarray(np.asarray(inputs["centers"], dtype=np.float32))
    assert x.shape == (B, D) and labels.shape == (B,) and centers.shape == (C, D)

    if _NC is None:
        _NC = build_nc()
    in_maps, los = _shard(x, labels, centers)
    res = run_bass_kernel_spmd(_NC, in_maps, core_ids=list(range(N_CORES)))
    return combine_partials([r["partial"] for r in res.results], los, labels)
